# revision 3
# baseline (speedup 1.0000x reference)
"""Cross-attention kernel for Trainium2, sharded over 8 NeuronCores.

Problem (hardcoded): B=2, N=M=2048, query/context dim 1024, 8 heads x 64.
Sharding: core c -> (batch b=c//4, head-pair hp=c%4). Each core projects
q/k/v for its 2 heads (column-parallel), runs attention for those heads,
and computes a partial output projection (row-parallel over Wo). The host
sums the 4 partials per batch (bf16) and adds the bias.

Schedule: the ScalarE exp stream (64 x [128,1024] ACTIVATEs ~ 73us) is the
hard floor; everything is arranged around keeping it saturated:
  - warmup matmuls release the HAM clock gate before real work
  - per context-window: k/q/vT projections, then attention chunks for TWO
    query windows interleave with the next window's projections
  - attention inner loop is software-pipelined (sim(t+1) emitted before
    attnV(t)) so the PE never head-of-line blocks the exp stream
  - v projection computed directly transposed (ctx chunk as stationary)
  - v3 layout [dims | ones] puts S at accumulator row 64 -> only one
    SBUF->SBUF lane-shift DMA per query window (head B)
  - output written bf16, batched DMAs
"""

import numpy as np
import ml_dtypes

B = 2
N = 2048  # query tokens per batch
M = 2048  # context tokens per batch
D = 1024  # query/context feature dim
HEADS = 8
DH = 64
INNER = 512
SCALE = DH**-0.5
P = 128
TW = 512  # token window
NKC = D // P  # contraction chunks for projections (8)
NW = M // TW  # context/query windows (4)
NJT = M // P  # key tiles (16)

_STATE = {}


def _build_nc():
    import concourse.bacc as bacc
    import concourse.tile as tile
    import concourse.mybir as mybir
    from concourse.masks import make_identity

    dt = mybir.dt
    bf16 = dt.bfloat16
    f32 = dt.float32

    nc = bacc.Bacc("TRN2", target_bir_lowering=False, debug=False)

    xT = nc.dram_tensor("xT", [NKC, P, N], bf16, kind="ExternalInput").ap()
    ctxT = nc.dram_tensor("ctxT", [NKC, P, M], bf16, kind="ExternalInput").ap()
    wq = nc.dram_tensor("wq", [P, NKC, P], bf16, kind="ExternalInput").ap()
    wk = nc.dram_tensor("wk", [P, NKC, P], bf16, kind="ExternalInput").ap()
    wv = nc.dram_tensor("wv", [P, NKC, P], bf16, kind="ExternalInput").ap()
    wo = nc.dram_tensor("wo", [P, 2, 512], bf16, kind="ExternalInput").ap()
    # output blocks: row r = blk*128+p, col = fc*512+c
    outp = nc.dram_tensor("outp", [16, P, 2, 512], bf16, kind="ExternalOutput").ap()

    with tile.TileContext(nc) as tc:
        with (
            tc.tile_pool(name="const", bufs=1) as constp,
            tc.tile_pool(name="weights", bufs=1) as wpool,
            tc.tile_pool(name="persist", bufs=1) as persist,
            tc.tile_pool(name="attn", bufs=4) as apool,
            tc.tile_pool(name="evict", bufs=4) as epool,
            tc.tile_pool(name="norm", bufs=2) as npool,
            tc.tile_pool(name="stage", bufs=2) as spool,
            tc.tile_pool(name="psum_sim", bufs=2, space="PSUM") as psum_sim,
            tc.tile_pool(name="psum_acc", bufs=4, space="PSUM") as psum_acc,
        ):
            identity = constp.tile([P, P], bf16)
            make_identity(nc, identity)
            onesP = constp.tile([P, 64], bf16)
            nc.vector.memset(onesP[:], 1.0)
            junk = constp.tile([P, TW], bf16)
            nc.vector.memset(junk[:], 0.0)

            # ---- weights via SWDGE (parallel with input stream on HWDGE) ----
            wk_sb = wpool.tile([P, NKC, P], bf16)
            nc.gpsimd.dma_start(wk_sb[:], wk[:])
            wv_sb = wpool.tile([P, NKC, P], bf16)
            nc.gpsimd.dma_start(wv_sb[:], wv[:])
            wq_sb = wpool.tile([P, NKC, P], bf16)
            nc.gpsimd.dma_start(wq_sb[:], wq[:])
            wo_sb = wpool.tile([P, 2, 512], bf16)
            nc.gpsimd.dma_start(wo_sb[:], wo[:])

            # ---- inputs: one batched DMA per window ----
            ctx_sb = persist.tile([P, NKC, M], bf16)
            x_sb = persist.tile([P, NKC, N], bf16)
            for w in range(NW):
                wsl = slice(w * TW, (w + 1) * TW)
                nc.sync.dma_start(
                    ctx_sb[:, :, wsl], ctxT[:, :, wsl].transpose([1, 0, 2])
                )
                nc.sync.dma_start(
                    x_sb[:, :, wsl], xT[:, :, wsl].transpose([1, 0, 2])
                )

            # ---- HAM warmup: ~3.4us of junk matmuls while DMAs stream ----
            wu = psum_sim.tile([P, TW], f32, tag="sim")
            for _ in range(8):
                nc.tensor.matmul(wu[:], identity[:], junk[:], start=True, stop=True)

            # per-window persistent k (transposed) and v (natural + ones col)
            # v3 layout per head: [64 dims | ones] -> S lands at acc row 64
            kTw = [persist.tile([P, TW], bf16, name=f"kTw{w}", tag=f"kTw{w}") for w in range(NW)]
            v3w = [persist.tile([P, TW // P, 130], bf16, name=f"v3w{w}", tag=f"v3w{w}") for w in range(NW)]
            qws = [persist.tile([P, TW], bf16, name=f"qw{w}", tag=f"qw{w}") for w in range(NW)]
            for w in range(NW):
                nc.vector.memset(v3w[w][:, :, 64:65], 1.0)
                nc.vector.memset(v3w[w][:, :, 129:130], 1.0)

            def proj_window(w):
                wsl = slice(w * TW, (w + 1) * TW)
                # k projection (transposed layout: [dims, keys])
                psk = psum_sim.tile([P, TW], f32, tag="sim")
                for kc in range(NKC):
                    nc.tensor.matmul(
                        psk[:], wk_sb[:, kc, :], ctx_sb[:, kc, wsl],
                        start=(kc == 0), stop=(kc == NKC - 1),
                    )
                nc.vector.tensor_copy(kTw[w][:], psk[:])
                # q projection for query window iw=w
                psq = psum_sim.tile([P, TW], f32, tag="sim")
                for kc in range(NKC):
                    nc.tensor.matmul(
                        psq[:], wq_sb[:, kc, :], x_sb[:, kc, wsl],
                        start=(kc == 0), stop=(kc == NKC - 1),
                    )
                nc.vector.tensor_copy(qws[w][:], psq[:])
                # v projection, directly transposed: [keys, dims]
                vt = psum_sim.tile([P, TW // P, P], f32, tag="sim")
                for t in range(TW // P):
                    ksl = slice(w * TW + t * P, w * TW + (t + 1) * P)
                    for kc in range(NKC):
                        nc.tensor.matmul(
                            vt[:, t, :], ctx_sb[:, kc, ksl], wv_sb[:, kc, :],
                            start=(kc == 0), stop=(kc == NKC - 1),
                        )
                nc.vector.tensor_copy(v3w[w][:, :, 0:64], vt[:, :, 0:64])
                nc.vector.tensor_copy(v3w[w][:, :, 65:129], vt[:, :, 64:128])

            # o accumulators per live query window: [65, TW] (rows 0-63 = o,
            # row 64 = S from the ones column), one bank each, 2 windows live
            o_ps = {}
            pending = []  # software pipeline: attnV lags sim/exp by one tile

            def flush_pending():
                while pending:
                    iw, w, t, a2, first, last = pending.pop(0)
                    nc.tensor.matmul(
                        o_ps[iw][0][:], v3w[w][:, t, 0:65], a2[:, 0:TW],
                        start=first, stop=last, skip_group_check=True,
                    )
                    nc.tensor.matmul(
                        o_ps[iw][1][:], v3w[w][:, t, 65:130], a2[:, TW:],
                        start=first, stop=last, skip_group_check=True,
                    )

            def attn_chunk(iw, w):
                if iw not in o_ps:
                    o_ps[iw] = (
                        psum_acc.tile([65, TW], f32, name=f"oA{iw}", tag="acc"),
                        psum_acc.tile([65, TW], f32, name=f"oB{iw}", tag="acc"),
                    )
                for t in range(TW // P):
                    jt = w * (TW // P) + t
                    jsl = slice(t * P, (t + 1) * P)
                    s2 = psum_sim.tile([P, 2 * TW], f32, tag="sim")
                    nc.tensor.matmul(
                        s2[:, 0:TW], kTw[w][0:64, jsl], qws[iw][0:64, :],
                        skip_group_check=True,
                    )
                    nc.tensor.matmul(
                        s2[:, TW:], kTw[w][64:128, jsl], qws[iw][64:128, :],
                        skip_group_check=True,
                    )
                    flush_pending()
                    a2 = apool.tile([P, 2 * TW], bf16, tag="a")
                    nc.scalar.activation(
                        a2[:], s2[:], mybir.ActivationFunctionType.Exp, scale=SCALE
                    )
                    pending.append((iw, w, t, a2, jt == 0, jt == NJT - 1))

            def finalize(iw):
                flush_pending()
                oA, oB = o_ps.pop(iw)
                evA = epool.tile([65, TW], bf16, tag="ev")
                nc.vector.tensor_copy(evA[:], oA[:])
                evB = epool.tile([65, TW], bf16, tag="ev")
                nc.vector.tensor_copy(evB[:], oB[:])
                # broadcast S across partitions: rows 0-63 = S_A, 64-127 = S_B
                bc_ps = psum_sim.tile([P, TW], f32, tag="sim")
                nc.tensor.matmul(bc_ps[0:64, :], onesP[64:65, :], evA[64:65, :])
                nc.tensor.matmul(bc_ps[64:128, :], onesP[64:65, :], evB[64:65, :])
                rbc = npool.tile([P, TW], f32, tag="rbc")
                nc.vector.reciprocal_approx_fast(rbc[:], bc_ps[:])
                # head B lane-shift into rows 64-127; head A already aligned
                ao_u = npool.tile([P, TW], bf16, tag="aou")
                nc.sync.dma_start(ao_u[64:128, :], evB[0:64, :])
                ao = npool.tile([P, TW], bf16, tag="ao")
                nc.vector.tensor_mul(ao[0:64, :], evA[0:64, :], rbc[0:64, :])
                nc.vector.tensor_mul(ao[64:128, :], ao_u[64:128, :], rbc[64:128, :])
                # partial output projection, staged then one DMA per 128 rows
                stage = spool.tile([P, 4, 2, 512], bf16, tag="st")
                for it in range(TW // P):
                    for fc in range(2):
                        op_ps = psum_sim.tile([P, 512], f32, tag="sim")
                        nc.tensor.matmul(
                            op_ps[:], ao[:, it * P : (it + 1) * P], wo_sb[:, fc, :]
                        )
                        nc.vector.tensor_copy(stage[:, it, fc, :], op_ps[:])
                for it in range(TW // P):
                    nc.gpsimd.dma_start(outp[iw * 4 + it], stage[:, it, :, :])

            # ---- schedule: saturate ScalarE from the first window on ----
            proj_window(0)
            attn_chunk(0, 0)
            proj_window(1)
            attn_chunk(1, 0)
            attn_chunk(0, 1)
            proj_window(2)
            attn_chunk(1, 1)
            attn_chunk(0, 2)
            proj_window(3)
            attn_chunk(1, 2)
            attn_chunk(0, 3)
            finalize(0)
            attn_chunk(1, 3)
            finalize(1)
            for iw in (2, 3):
                for w in range(NW):
                    attn_chunk(iw, w)
                finalize(iw)

    nc.compile()
    return nc


def _get_nc():
    if "nc" not in _STATE:
        _STATE["nc"] = _build_nc()
    return _STATE["nc"]


def _make_in_maps(x, context, Wq, Wk, Wv, Wo):
    bf = ml_dtypes.bfloat16

    def wslice(W, hp):
        # [1024, 128] -> [p, kc, m] with k = kc*128 + p
        s = W[:, hp * P : (hp + 1) * P]
        return np.ascontiguousarray(
            s.reshape(NKC, P, P).transpose(1, 0, 2)
        ).astype(bf)

    xTs = [
        np.ascontiguousarray(x[b].T).astype(bf).reshape(NKC, P, N) for b in range(B)
    ]
    cTs = [
        np.ascontiguousarray(context[b].T).astype(bf).reshape(NKC, P, M)
        for b in range(B)
    ]
    in_maps = []
    for c in range(8):
        b, hp = c // 4, c % 4
        in_maps.append(
            {
                "xT": xTs[b],
                "ctxT": cTs[b],
                "wq": wslice(Wq, hp),
                "wk": wslice(Wk, hp),
                "wv": wslice(Wv, hp),
                "wo": np.ascontiguousarray(
                    Wo[hp * P : (hp + 1) * P, :].reshape(P, 2, 512)
                ).astype(bf),
            }
        )
    return in_maps


def kernel(x, context, Wq, Wk, Wv, Wo, bo, _spmd_kwargs=None):
    from concourse.bass_utils import run_bass_kernel_spmd

    nc = _get_nc()
    in_maps = _make_in_maps(x, context, Wq, Wk, Wv, Wo)
    res = run_bass_kernel_spmd(
        nc, in_maps, core_ids=list(range(8)), **(_spmd_kwargs or {})
    )
    _STATE["last_result"] = res
    outs = [
        np.asarray(r["outp"]).astype(np.float32).reshape(N, D) for r in res.results
    ]
    out = np.empty((B, N, D), np.float32)
    for b in range(B):
        out[b] = outs[4 * b] + outs[4 * b + 1] + outs[4 * b + 2] + outs[4 * b + 3]
        out[b] += bo.astype(np.float32)
    return out


# revision 6
# speedup vs baseline: 1.1127x; 1.1127x over previous
"""Cross-attention kernel for Trainium2, sharded over 8 NeuronCores.

Problem (hardcoded): B=2, N=M=2048, query/context dim 1024, 8 heads x 64.
Sharding: core c -> (batch b=c//4, head-pair hp=c%4). Each core projects
q/k/v for its 2 heads (column-parallel), runs attention for those heads,
and computes a partial output projection (row-parallel over Wo). The host
sums the 4 partials per batch (bf16) and adds the bias.

The ScalarE exp stream (64 x [128,1024] ACTIVATEs ~ 73us) is the hard
floor; the schedule keeps it saturated from ~8us on:
  - weights land first on the HWDGE FIFO, then ctx/x windows in need order
  - warmup matmuls release the HAM clock gate before real work
  - attention is processed in 4-key-tile chunks, window-major across ALL
    four query windows; per-chunk PSUM accumulators are merged into SBUF
    f32 accumulators by the DVE, so PSUM never holds long-lived state and
    chunks from any query window can interleave
  - the inner loop is software-pipelined (attnV lags sim/exp by 2 tiles)
    so the in-order PE queue never head-of-line blocks the exp stream
  - k/q/v projections are spliced between chunks as PE filler
  - v projection computed directly transposed (ctx chunk as stationary)
  - v3 layout [dims | ones] puts S at accumulator row 64 -> only one
    SBUF->SBUF lane-shift DMA per query window (head B)
  - output written bf16, batched DMAs
"""

import numpy as np
import ml_dtypes

B = 2
N = 2048  # query tokens per batch
M = 2048  # context tokens per batch
D = 1024  # query/context feature dim
HEADS = 8
DH = 64
INNER = 512
SCALE = DH**-0.5
P = 128
TW = 512  # token window
NKC = D // P  # contraction chunks for projections (8)
NW = M // TW  # context/query windows (4)
NT = TW // P  # key tiles per window (4)

_STATE = {}


def _build_nc():
    import concourse.bacc as bacc
    import concourse.tile as tile
    import concourse.mybir as mybir
    from concourse.masks import make_identity

    dt = mybir.dt
    bf16 = dt.bfloat16
    f32 = dt.float32

    nc = bacc.Bacc("TRN2", target_bir_lowering=False, debug=False)

    xT = nc.dram_tensor("xT", [NKC, P, N], bf16, kind="ExternalInput").ap()
    ctxT = nc.dram_tensor("ctxT", [NKC, P, M], bf16, kind="ExternalInput").ap()
    wq = nc.dram_tensor("wq", [P, NKC, P], bf16, kind="ExternalInput").ap()
    wk = nc.dram_tensor("wk", [P, NKC, P], bf16, kind="ExternalInput").ap()
    wv = nc.dram_tensor("wv", [P, NKC, P], bf16, kind="ExternalInput").ap()
    wo = nc.dram_tensor("wo", [P, 2, 512], bf16, kind="ExternalInput").ap()
    # output blocks: row r = blk*128+p, col = fc*512+c
    outp = nc.dram_tensor("outp", [16, P, 2, 512], bf16, kind="ExternalOutput").ap()

    with tile.TileContext(nc) as tc:
        with (
            tc.tile_pool(name="const", bufs=1) as constp,
            tc.tile_pool(name="weights", bufs=1) as wpool,
            tc.tile_pool(name="persist", bufs=1) as persist,
            tc.tile_pool(name="attn", bufs=6) as apool,
            tc.tile_pool(name="evict", bufs=4) as epool,
            tc.tile_pool(name="norm", bufs=2) as npool,
            tc.tile_pool(name="stage", bufs=2) as spool,
            tc.tile_pool(name="psum_sim", bufs=2, space="PSUM") as psum_sim,
            tc.tile_pool(name="psum_work", bufs=2, space="PSUM") as psum_work,
        ):
            identity = constp.tile([P, P], bf16)
            make_identity(nc, identity)
            onesF = constp.tile([P, 64], f32)
            nc.vector.memset(onesF[:], 1.0)
            junk = constp.tile([P, TW], bf16)
            nc.vector.memset(junk[:], 0.0)

            # ---- weights FIRST on the HWDGE FIFO (small, unblock projections) ----
            wk_sb = wpool.tile([P, NKC, P], bf16)
            nc.sync.dma_start(wk_sb[:], wk[:])
            wq_sb = wpool.tile([P, NKC, P], bf16)
            nc.sync.dma_start(wq_sb[:], wq[:])
            wv_sb = wpool.tile([P, NKC, P], bf16)
            nc.sync.dma_start(wv_sb[:], wv[:])
            wo_sb = wpool.tile([P, 2, 512], bf16)
            nc.sync.dma_start(wo_sb[:], wo[:])

            # ---- inputs: one batched DMA per window, in consumption order ----
            ctx_sb = persist.tile([P, NKC, M], bf16)
            x_sb = persist.tile([P, NKC, N], bf16)

            def load(dst, src, w):
                wsl = slice(w * TW, (w + 1) * TW)
                nc.sync.dma_start(dst[:, :, wsl], src[:, :, wsl].transpose([1, 0, 2]))

            load(ctx_sb, ctxT, 0)
            load(x_sb, xT, 0)
            load(x_sb, xT, 1)
            load(ctx_sb, ctxT, 1)
            load(x_sb, xT, 2)
            load(ctx_sb, ctxT, 2)
            load(x_sb, xT, 3)
            load(ctx_sb, ctxT, 3)

            # ---- HAM warmup: ~4us of junk matmuls while DMAs stream ----
            wu = psum_sim.tile([P, TW], f32, tag="sim")
            for _ in range(10):
                nc.tensor.matmul(wu[:], identity[:], junk[:], start=True, stop=True)

            # per-window persistent k (transposed) and v (natural + ones col)
            # v3 layout per head: [64 dims | ones] -> S lands at acc row 64
            kTw = [persist.tile([P, TW], bf16, name=f"kTw{w}", tag=f"kTw{w}") for w in range(NW)]
            v3w = [persist.tile([P, NT, 130], bf16, name=f"v3w{w}", tag=f"v3w{w}") for w in range(NW)]
            qws = [persist.tile([P, TW], bf16, name=f"qw{w}", tag=f"qw{w}") for w in range(NW)]
            # f32 output accumulators per query window: rows 0-63 = o, 64 = S
            o_sb = [
                persist.tile([65, 2, TW], f32, name=f"osb{iw}", tag=f"osb{iw}")
                for iw in range(NW)
            ]
            for w in range(NW):
                nc.vector.memset(v3w[w][:, :, 64:65], 1.0)
                nc.vector.memset(v3w[w][:, :, 129:130], 1.0)

            def proj_k(w):
                wsl = slice(w * TW, (w + 1) * TW)
                psk = psum_work.tile([P, TW], f32, tag="work")
                for kc in range(NKC):
                    nc.tensor.matmul(
                        psk[:], wk_sb[:, kc, :], ctx_sb[:, kc, wsl],
                        start=(kc == 0), stop=(kc == NKC - 1),
                    )
                nc.vector.tensor_copy(kTw[w][:], psk[:])

            def proj_q(w):
                wsl = slice(w * TW, (w + 1) * TW)
                psq = psum_work.tile([P, TW], f32, tag="work")
                for kc in range(NKC):
                    nc.tensor.matmul(
                        psq[:], wq_sb[:, kc, :], x_sb[:, kc, wsl],
                        start=(kc == 0), stop=(kc == NKC - 1),
                    )
                nc.vector.tensor_copy(qws[w][:], psq[:])

            def proj_v(w):
                # directly transposed: [keys, dims], ctx chunk stationary
                vt = psum_work.tile([P, NT, P], f32, tag="work")
                for t in range(NT):
                    ksl = slice(w * TW + t * P, w * TW + (t + 1) * P)
                    for kc in range(NKC):
                        nc.tensor.matmul(
                            vt[:, t, :], ctx_sb[:, kc, ksl], wv_sb[:, kc, :],
                            start=(kc == 0), stop=(kc == NKC - 1),
                        )
                nc.vector.tensor_copy(v3w[w][:, :, 0:64], vt[:, :, 0:64])
                nc.vector.tensor_copy(v3w[w][:, :, 65:129], vt[:, :, 64:128])

            # per-chunk psum accumulators, merged to o_sb after each chunk
            chunk_ps = {}
            pending = []

            def SE(iw, w, t):
                jsl = slice(t * P, (t + 1) * P)
                s2 = psum_sim.tile([P, 2 * TW], f32, tag="sim")
                nc.tensor.matmul(
                    s2[:, 0:TW], kTw[w][0:64, jsl], qws[iw][0:64, :],
                    skip_group_check=True,
                )
                nc.tensor.matmul(
                    s2[:, TW:], kTw[w][64:128, jsl], qws[iw][64:128, :],
                    skip_group_check=True,
                )
                a2 = apool.tile([P, 2 * TW], bf16, tag="a")
                nc.scalar.activation(
                    a2[:], s2[:], mybir.ActivationFunctionType.Exp, scale=SCALE
                )
                pending.append((iw, w, t, a2))

            def F():
                iw, w, t, a2 = pending.pop(0)
                if (iw, w) not in chunk_ps:
                    chunk_ps[(iw, w)] = psum_work.tile(
                        [65, 2, TW], f32, name=f"cp{iw}_{w}", tag="work"
                    )
                cp = chunk_ps[(iw, w)]
                nc.tensor.matmul(
                    cp[:, 0, :], v3w[w][:, t, 0:65], a2[:, 0:TW],
                    start=(t == 0), stop=(t == NT - 1), skip_group_check=True,
                )
                nc.tensor.matmul(
                    cp[:, 1, :], v3w[w][:, t, 65:130], a2[:, TW:],
                    start=(t == 0), stop=(t == NT - 1), skip_group_check=True,
                )

            def MERGE(iw, w):
                cp = chunk_ps.pop((iw, w))
                for h in range(2):
                    if w == 0:
                        nc.vector.tensor_copy(o_sb[iw][:, h, :], cp[:, h, :])
                    else:
                        nc.vector.tensor_add(
                            o_sb[iw][:, h, :], o_sb[iw][:, h, :], cp[:, h, :]
                        )

            aos = {}

            def NORM(iw):
                # broadcast 1/S across partitions: rows 0-63 <- S_A, 64-127 <- S_B
                bc_ps = psum_work.tile([P, TW], f32, tag="work")
                nc.tensor.matmul(bc_ps[0:64, :], onesF[64:65, :], o_sb[iw][64:65, 0, :])
                nc.tensor.matmul(bc_ps[64:128, :], onesF[64:65, :], o_sb[iw][64:65, 1, :])
                rbc = npool.tile([P, TW], f32, tag="rbc")
                nc.vector.reciprocal_approx_fast(rbc[:], bc_ps[:])
                # head B lane-shift into rows 64-127; head A already aligned
                evB = epool.tile([64, TW], bf16, tag="ev")
                nc.vector.tensor_copy(evB[:], o_sb[iw][0:64, 1, :])
                ao_u = npool.tile([P, TW], bf16, tag="aou")
                nc.sync.dma_start(ao_u[64:128, :], evB[:])
                ao = npool.tile([P, TW], bf16, tag="ao")
                nc.vector.tensor_mul(ao[0:64, :], o_sb[iw][0:64, 0, :], rbc[0:64, :])
                nc.vector.tensor_mul(ao[64:128, :], ao_u[64:128, :], rbc[64:128, :])
                aos[iw] = ao

            def OPROJ(iw):
                ao = aos.pop(iw)
                stage = spool.tile([P, 4, 2, 512], bf16, tag="st")
                for it in range(NT):
                    for fc in range(2):
                        op_ps = psum_work.tile([P, 512], f32, tag="work")
                        nc.tensor.matmul(
                            op_ps[:], ao[:, it * P : (it + 1) * P], wo_sb[:, fc, :]
                        )
                        nc.vector.tensor_copy(stage[:, it, fc, :], op_ps[:])
                for it in range(NT):
                    nc.gpsimd.dma_start(outp[iw * 4 + it], stage[:, it, :, :])

            # ---- schedule: window-major chunks, PE fillers spliced in ----
            # pre-fillers run before the chunk's first sim; mid-fillers run
            # after MERGE of the previous chunk (so they see its results and
            # a freed psum_work slot) but before the chunk's 3rd key tile,
            # whose flush is the first attnV that may need a new v3 window.
            chunks = [
                (0, 0), (1, 0), (0, 1), (1, 1),
                (2, 0), (3, 0), (2, 1), (3, 1),
                (0, 2), (1, 2), (2, 2), (3, 2),
                (0, 3), (1, 3), (2, 3), (3, 3),
            ]
            pre = {
                0: [lambda: proj_k(0), lambda: proj_q(0)],
                1: [lambda: proj_q(1)],
                2: [lambda: proj_k(1)],
                4: [lambda: proj_q(2)],
                5: [lambda: proj_q(3)],
                7: [lambda: proj_k(2)],
                11: [lambda: proj_k(3)],
            }
            mid = {
                2: [lambda: proj_v(1)],
                8: [lambda: proj_v(2)],
                12: [lambda: proj_v(3)],
                13: [lambda: NORM(0)],
                14: [lambda: NORM(1), lambda: OPROJ(0)],
                15: [lambda: NORM(2), lambda: OPROJ(1)],
            }
            for ci, (iw, w) in enumerate(chunks):
                for f in pre.get(ci, []):
                    f()
                SE(iw, w, 0)
                if ci == 0:
                    SE(iw, w, 1)
                    proj_v(0)
                    F()
                    F()
                    SE(iw, w, 2)
                    SE(iw, w, 3)
                else:
                    F()
                    SE(iw, w, 1)
                    F()
                    MERGE(*chunks[ci - 1])
                    for f in mid.get(ci, []):
                        f()
                    SE(iw, w, 2)
                    F()
                    SE(iw, w, 3)
                    F()
            F()
            F()
            MERGE(*chunks[15])
            OPROJ(2)
            NORM(3)
            OPROJ(3)

    nc.compile()
    return nc


def _get_nc():
    if "nc" not in _STATE:
        _STATE["nc"] = _build_nc()
    return _STATE["nc"]


def _make_in_maps(x, context, Wq, Wk, Wv, Wo):
    bf = ml_dtypes.bfloat16

    def wslice(W, hp):
        # [1024, 128] -> [p, kc, m] with k = kc*128 + p
        s = W[:, hp * P : (hp + 1) * P]
        return np.ascontiguousarray(
            s.reshape(NKC, P, P).transpose(1, 0, 2)
        ).astype(bf)

    xTs = [
        np.ascontiguousarray(x[b].T).astype(bf).reshape(NKC, P, N) for b in range(B)
    ]
    cTs = [
        np.ascontiguousarray(context[b].T).astype(bf).reshape(NKC, P, M)
        for b in range(B)
    ]
    in_maps = []
    for c in range(8):
        b, hp = c // 4, c % 4
        in_maps.append(
            {
                "xT": xTs[b],
                "ctxT": cTs[b],
                "wq": wslice(Wq, hp),
                "wk": wslice(Wk, hp),
                "wv": wslice(Wv, hp),
                "wo": np.ascontiguousarray(
                    Wo[hp * P : (hp + 1) * P, :].reshape(P, 2, 512)
                ).astype(bf),
            }
        )
    return in_maps


def kernel(x, context, Wq, Wk, Wv, Wo, bo, _spmd_kwargs=None):
    from concourse.bass_utils import run_bass_kernel_spmd

    nc = _get_nc()
    in_maps = _make_in_maps(x, context, Wq, Wk, Wv, Wo)
    res = run_bass_kernel_spmd(
        nc, in_maps, core_ids=list(range(8)), **(_spmd_kwargs or {})
    )
    _STATE["last_result"] = res
    outs = [
        np.asarray(r["outp"]).astype(np.float32).reshape(N, D) for r in res.results
    ]
    out = np.empty((B, N, D), np.float32)
    for b in range(B):
        out[b] = outs[4 * b] + outs[4 * b + 1] + outs[4 * b + 2] + outs[4 * b + 3]
        out[b] += bo.astype(np.float32)
    return out
